# revision 8
# baseline (speedup 1.0000x reference)
"""Walsh-Hadamard transform (4096-point, orthonormal) on trn2, 8 cores.

y[r] = (H_4096 @ x[r]) / 64  for each of 16384 rows.

Scheme: H_4096 = H_16 (x) H_2 (x) H_128 over n = i*256 + v*128 + u
(i in 16, v in 2, u in 128). Rows are processed in groups of 8. An SBUF
tile holds an 8-row group as [128 partitions = (rr*16 + i), 256 free =
(v,u)]; each partition row is one contiguous 1 KiB chunk of DRAM (256
f32), which keeps DMA descriptors at full HBM-side efficiency.

Loads/stores are plain HWDGE f32. mm1 runs in fp32; mm2 runs in bf16
(the PSUM->SBUF mid-copy casts; all Hadamard factors are exactly
representable: +-1 and +-2^-6), accumulation in fp32 PSUM. Two matmul
stages per group:
  mm1 (x2, one per v): out1_v = Xslice_v.T @ BD   (BD = I_8 (x) H_16)
      -> [u, (rr,a)] in PSUM; the data is the stationary operand so the
      matmul also performs the layout corner-turn.
  mm2 (x2, accumulating, N=256): ps2 += t1_v.T @ M_v with
      M_0 = [Hs | Hs], M_1 = [Hs | -Hs], Hs = H_128/64
      -> [(rr,a), (v',u')] which is exactly the natural row-major output
      layout, so the store is also plain 1 KiB-chunk DMAs.

Work is sharded row-wise: core c processes rows [c*2048, (c+1)*2048).
"""

import numpy as np

N_ROWS = 16384
DIM = 4096
N_CORES = 8
R_PER_CORE = N_ROWS // N_CORES  # 2048

G = 8  # 8-row groups per DMA chunk -> 64 rows = 1 MiB per direction
SB = 2  # groups per PSUM bank (2 * 256 fp32 = 512 = one bank)

_PROG_CACHE = {}


def _hadamard(n: int) -> np.ndarray:
    H = np.array([[1.0]], dtype=np.float64)
    while H.shape[0] < n:
        H = np.block([[H, H], [H, -H]])
    return H


def _build_program():
    import concourse.mybir as mybir
    from concourse import bacc
    from concourse.tile import TileContext

    f32 = mybir.dt.float32
    bf16 = mybir.dt.bfloat16
    nc = bacc.Bacc("TRN2")

    x = nc.declare_dram_parameter("x", [R_PER_CORE, DIM], f32, isOutput=False)
    y = nc.declare_dram_parameter("y", [R_PER_CORE, DIM], f32, isOutput=True)

    BD = np.kron(np.eye(8), _hadamard(16)).astype(np.float32)  # [(rr,i),(rr,a)]
    Hs = _hadamard(128) / 64.0  # [u, u']
    M0 = np.concatenate([Hs, Hs], axis=1).astype(np.float32)  # [u, (v',u')]
    M1 = np.concatenate([Hs, -Hs], axis=1).astype(np.float32)

    bd_d = nc.inline_tensor(BD.astype(np.float32), "bd_const")
    m0_d = nc.inline_tensor(M0, "m0_const")
    m1_d = nc.inline_tensor(M1, "m1_const")

    n_chunks = R_PER_CORE // (8 * G)  # 32

    xv = x[:].rearrange("(cb g rr) (i jj) -> cb (rr i) g jj", g=G, rr=8, i=16, jj=256)
    yv = y[:].rearrange("(cb g rr) (a jj) -> cb (rr a) g jj", g=G, rr=8, a=16, jj=256)

    with TileContext(nc) as tc:
        with (
            tc.tile_pool(name="consts", bufs=1) as cpool,
            tc.tile_pool(name="inp", bufs=4) as inpool,
            tc.tile_pool(name="outp", bufs=3) as outpool,
            tc.tile_pool(name="mid", bufs=6) as midpool,
            tc.tile_pool(name="ps1", bufs=3, space="PSUM") as ps1pool,
            tc.tile_pool(name="ps2", bufs=3, space="PSUM") as ps2pool,
        ):
            bd_f = cpool.tile([128, 128], f32)
            m0_f = cpool.tile([128, 256], f32)
            m1_f = cpool.tile([128, 256], f32)
            nc.sync.dma_start(out=bd_f[:], in_=bd_d[:])
            nc.sync.dma_start(out=m0_f[:], in_=m0_d[:])
            nc.sync.dma_start(out=m1_f[:], in_=m1_d[:])
            m0_sb = cpool.tile([128, 256], bf16)
            m1_sb = cpool.tile([128, 256], bf16)
            nc.vector.tensor_copy(out=m0_sb[:], in_=m0_f[:])
            nc.vector.tensor_copy(out=m1_sb[:], in_=m1_f[:])

            for cb in range(n_chunks):
                in_tile = inpool.tile([128, G, 256], f32)
                nc.sync.dma_start(out=in_tile[:], in_=xv[cb])
                out_tile = outpool.tile([128, G, 256], f32)
                for s in range(G // SB):
                    ps1 = ps1pool.tile([128, SB * 256], f32)
                    for g2 in range(SB):
                        g = s * SB + g2
                        for v in range(2):
                            nc.tensor.matmul(
                                ps1[:, g2 * 256 + v * 128 : g2 * 256 + (v + 1) * 128],
                                in_tile[:, g, v * 128 : (v + 1) * 128],
                                bd_f[:],
                                start=True,
                                stop=True,
                            )
                    t1 = midpool.tile([128, SB * 256], bf16)
                    nc.scalar.copy(t1[:], ps1[:])
                    ps2 = ps2pool.tile([128, SB * 256], f32)
                    for g2 in range(SB):
                        nc.tensor.matmul(
                            ps2[:, g2 * 256 : (g2 + 1) * 256],
                            t1[:, g2 * 256 : g2 * 256 + 128],
                            m0_sb[:],
                            start=True,
                            stop=False,
                        )
                        nc.tensor.matmul(
                            ps2[:, g2 * 256 : (g2 + 1) * 256],
                            t1[:, g2 * 256 + 128 : g2 * 256 + 256],
                            m1_sb[:],
                            start=False,
                            stop=True,
                        )
                    nc.vector.tensor_copy(
                        out=out_tile[:, s * SB : (s + 1) * SB].rearrange(
                            "p g c -> p (g c)"
                        ),
                        in_=ps2[:],
                    )
                nc.sync.dma_start(out=yv[cb], in_=out_tile[:])

    nc.compile()
    return nc


def _get_program():
    if "nc" not in _PROG_CACHE:
        _PROG_CACHE["nc"] = _build_program()
    return _PROG_CACHE["nc"]


def kernel(x, _trace=False, _trace_kwargs=None):
    from concourse.bass_utils import run_bass_kernel_spmd

    x = np.ascontiguousarray(np.asarray(x, dtype=np.float32))
    assert x.shape == (N_ROWS, DIM), x.shape

    nc = _get_program()
    core_ids = list(range(N_CORES))
    in_maps = [
        {"x": x[c * R_PER_CORE : (c + 1) * R_PER_CORE]} for c in core_ids
    ]
    res = run_bass_kernel_spmd(
        nc, in_maps, core_ids, trace=_trace, **(_trace_kwargs or {})
    )
    out = np.concatenate([r["y"] for r in res.results], axis=0)
    if _trace:
        return out, res
    return out


# revision 9
# speedup vs baseline: 1.0405x; 1.0405x over previous
"""Walsh-Hadamard transform (4096-point, orthonormal) on trn2, 8 cores.

y[r] = (H_4096 @ x[r]) / 64  for each of 16384 rows.

Scheme: H_4096 = H_16 (x) H_2 (x) H_128 over n = i*256 + v*128 + u
(i in 16, v in 2, u in 128). Rows are processed in groups of 8. An SBUF
tile holds an 8-row group as [128 partitions = (rr*16 + i), 256 free =
(v,u)]; each partition row is one contiguous 1 KiB chunk of DRAM (256
f32), which keeps DMA descriptors at full HBM-side efficiency.

Compute is in bf16 (the f32->bf16 cast happens inline in the SDMA
engines during both the load and the store, via SWDGE cast-DMA; all
Hadamard factors are exactly representable: +-1 and +-2^-6),
accumulation in fp32 PSUM. Two matmul stages per group:
  mm1 (x2, one per v): out1_v = Xslice_v.T @ BD   (BD = I_8 (x) H_16)
      -> [u, (rr,a)] in PSUM; the data is the stationary operand so the
      matmul also performs the layout corner-turn.
  mm2 (x2, accumulating, N=256): ps2 += t1_v.T @ M_v with
      M_0 = [Hs | Hs], M_1 = [Hs | -Hs], Hs = H_128/64
      -> [(rr,a), (v',u')] which is exactly the natural row-major output
      layout, so the store is also plain 1 KiB-chunk DMAs.

Work is sharded row-wise: core c processes rows [c*2048, (c+1)*2048).
"""

import numpy as np

N_ROWS = 16384
DIM = 4096
N_CORES = 8
R_PER_CORE = N_ROWS // N_CORES  # 2048

G = 8  # 8-row groups per DMA chunk -> 64 rows = 1 MiB per direction
SB = 2  # groups per PSUM bank (2 * 256 fp32 = 512 = one bank)

_PROG_CACHE = {}


def _hadamard(n: int) -> np.ndarray:
    H = np.array([[1.0]], dtype=np.float64)
    while H.shape[0] < n:
        H = np.block([[H, H], [H, -H]])
    return H


def _build_program():
    import concourse.mybir as mybir
    from concourse import bacc
    from concourse.tile import TileContext

    f32 = mybir.dt.float32
    bf16 = mybir.dt.bfloat16
    nc = bacc.Bacc("TRN2")

    x = nc.declare_dram_parameter("x", [R_PER_CORE, DIM], f32, isOutput=False)
    y = nc.declare_dram_parameter("y", [R_PER_CORE, DIM], f32, isOutput=True)

    BD = np.kron(np.eye(8), _hadamard(16)).astype(np.float32)  # [(rr,i),(rr,a)]
    Hs = _hadamard(128) / 64.0  # [u, u']
    M0 = np.concatenate([Hs, Hs], axis=1).astype(np.float32)  # [u, (v',u')]
    M1 = np.concatenate([Hs, -Hs], axis=1).astype(np.float32)

    bd_d = nc.inline_tensor(BD.astype(np.float32), "bd_const")
    m0_d = nc.inline_tensor(M0, "m0_const")
    m1_d = nc.inline_tensor(M1, "m1_const")

    n_chunks = R_PER_CORE // (8 * G)  # 32

    xv = x[:].rearrange("(cb g rr) (i jj) -> cb (rr i) g jj", g=G, rr=8, i=16, jj=256)
    yv = y[:].rearrange("(cb g rr) (a jj) -> cb (rr a) g jj", g=G, rr=8, a=16, jj=256)

    with TileContext(nc) as tc:
        with (
            tc.tile_pool(name="consts", bufs=1) as cpool,
            tc.tile_pool(name="inbf", bufs=4) as bfpool,
            tc.tile_pool(name="outp", bufs=3) as outpool,
            tc.tile_pool(name="mid", bufs=6) as midpool,
            tc.tile_pool(name="ps1", bufs=3, space="PSUM") as ps1pool,
            tc.tile_pool(name="ps2", bufs=3, space="PSUM") as ps2pool,
        ):
            bd_f = cpool.tile([128, 128], f32)
            m0_f = cpool.tile([128, 256], f32)
            m1_f = cpool.tile([128, 256], f32)
            nc.sync.dma_start(out=bd_f[:], in_=bd_d[:])
            nc.sync.dma_start(out=m0_f[:], in_=m0_d[:])
            nc.sync.dma_start(out=m1_f[:], in_=m1_d[:])
            bd_sb = cpool.tile([128, 128], bf16)
            nc.vector.tensor_copy(out=bd_sb[:], in_=bd_f[:])
            m0_sb = cpool.tile([128, 256], bf16)
            m1_sb = cpool.tile([128, 256], bf16)
            nc.vector.tensor_copy(out=m0_sb[:], in_=m0_f[:])
            nc.vector.tensor_copy(out=m1_sb[:], in_=m1_f[:])

            for cb in range(n_chunks):
                # SWDGE cast-during-DMA: reads f32 from HBM, lands bf16 in
                # SBUF (the cast runs inline in the SDMA engines).
                in_bf = bfpool.tile([128, G, 256], bf16)
                nc.gpsimd.dma_start(out=in_bf[:], in_=xv[cb])
                out_tile = outpool.tile([128, G, 256], bf16)
                for s in range(G // SB):
                    ps1 = ps1pool.tile([128, SB * 256], f32)
                    for g2 in range(SB):
                        g = s * SB + g2
                        for v in range(2):
                            nc.tensor.matmul(
                                ps1[:, g2 * 256 + v * 128 : g2 * 256 + (v + 1) * 128],
                                in_bf[:, g, v * 128 : (v + 1) * 128],
                                bd_sb[:],
                                start=True,
                                stop=True,
                            )
                    t1 = midpool.tile([128, SB * 256], bf16)
                    nc.scalar.copy(t1[:], ps1[:])
                    ps2 = ps2pool.tile([128, SB * 256], f32)
                    for g2 in range(SB):
                        nc.tensor.matmul(
                            ps2[:, g2 * 256 : (g2 + 1) * 256],
                            t1[:, g2 * 256 : g2 * 256 + 128],
                            m0_sb[:],
                            start=True,
                            stop=False,
                        )
                        nc.tensor.matmul(
                            ps2[:, g2 * 256 : (g2 + 1) * 256],
                            t1[:, g2 * 256 + 128 : g2 * 256 + 256],
                            m1_sb[:],
                            start=False,
                            stop=True,
                        )
                    nc.vector.tensor_copy(
                        out=out_tile[:, s * SB : (s + 1) * SB].rearrange(
                            "p g c -> p (g c)"
                        ),
                        in_=ps2[:],
                    )
                # SWDGE cast-during-DMA on the store too: bf16 in SBUF,
                # f32 in HBM (halves the SBUF-side S2M traffic).
                nc.gpsimd.dma_start(out=yv[cb], in_=out_tile[:])

    nc.compile()
    return nc


def _get_program():
    if "nc" not in _PROG_CACHE:
        _PROG_CACHE["nc"] = _build_program()
    return _PROG_CACHE["nc"]


def kernel(x, _trace=False, _trace_kwargs=None):
    from concourse.bass_utils import run_bass_kernel_spmd

    x = np.ascontiguousarray(np.asarray(x, dtype=np.float32))
    assert x.shape == (N_ROWS, DIM), x.shape

    nc = _get_program()
    core_ids = list(range(N_CORES))
    in_maps = [
        {"x": x[c * R_PER_CORE : (c + 1) * R_PER_CORE]} for c in core_ids
    ]
    res = run_bass_kernel_spmd(
        nc, in_maps, core_ids, trace=_trace, **(_trace_kwargs or {})
    )
    out = np.concatenate([r["y"] for r in res.results], axis=0)
    if _trace:
        return out, res
    return out


# revision 10
# speedup vs baseline: 1.2262x; 1.1785x over previous
"""Walsh-Hadamard transform (4096-point, orthonormal) on trn2, 8 cores.

y[r] = (H_4096 @ x[r]) / 64  for each of 16384 rows.

Scheme: H_4096 = H_16 (x) H_2 (x) H_128 over n = i*256 + v*128 + u
(i in 16, v in 2, u in 128). Rows are processed in groups of 8. An SBUF
tile holds an 8-row group as [128 partitions = (rr*16 + i), 256 free =
(v,u)]; each partition row is one contiguous 1 KiB chunk of DRAM (256
f32), which keeps DMA descriptors at full HBM-side efficiency.

Compute is in bf16 (the f32->bf16 cast happens inline in the SDMA
engines during both the load and the store, via SWDGE cast-DMA; all
Hadamard factors are exactly representable: +-1 and +-2^-6),
accumulation in fp32 PSUM. Two matmul stages per group:
  mm1 (x2, one per v): out1_v = Xslice_v.T @ BD   (BD = I_8 (x) H_16)
      -> [u, (rr,a)] in PSUM; the data is the stationary operand so the
      matmul also performs the layout corner-turn.
  mm2 (x2, accumulating, N=256): ps2 += t1_v.T @ M_v with
      M_0 = [Hs | Hs], M_1 = [Hs | -Hs], Hs = H_128/64
      -> [(rr,a), (v',u')] which is exactly the natural row-major output
      layout, so the store is also plain 1 KiB-chunk DMAs.

Work is sharded row-wise: core c processes rows [c*2048, (c+1)*2048).
"""

import numpy as np

N_ROWS = 16384
DIM = 4096
N_CORES = 8
R_PER_CORE = N_ROWS // N_CORES  # 2048

G = 8  # 8-row groups per DMA chunk -> 64 rows = 1 MiB per direction
SB = 2  # groups per PSUM bank (2 * 256 fp32 = 512 = one bank)

_PROG_CACHE = {}


def _hadamard(n: int) -> np.ndarray:
    H = np.array([[1.0]], dtype=np.float64)
    while H.shape[0] < n:
        H = np.block([[H, H], [H, -H]])
    return H


def _build_program():
    import concourse.mybir as mybir
    from concourse import bacc
    from concourse.tile import TileContext

    f32 = mybir.dt.float32
    bf16 = mybir.dt.bfloat16
    nc = bacc.Bacc("TRN2")

    x = nc.declare_dram_parameter("x", [R_PER_CORE, DIM], f32, isOutput=False)
    y = nc.declare_dram_parameter("y", [R_PER_CORE, DIM], f32, isOutput=True)

    BD = np.kron(np.eye(8), _hadamard(16)).astype(np.float32)  # [(rr,i),(rr,a)]
    Hs = _hadamard(128) / 64.0  # [u, u']
    M0 = np.concatenate([Hs, Hs], axis=1).astype(np.float32)  # [u, (v',u')]
    M1 = np.concatenate([Hs, -Hs], axis=1).astype(np.float32)

    bd_d = nc.inline_tensor(BD.astype(np.float32), "bd_const")
    m0_d = nc.inline_tensor(M0, "m0_const")
    m1_d = nc.inline_tensor(M1, "m1_const")

    n_chunks = R_PER_CORE // (8 * G)  # 32

    xv = x[:].rearrange("(cb g rr) (i jj) -> cb (rr i) g jj", g=G, rr=8, i=16, jj=256)
    yv = y[:].rearrange("(cb g rr) (a jj) -> cb (rr a) g jj", g=G, rr=8, a=16, jj=256)

    with TileContext(nc) as tc:
        with (
            tc.tile_pool(name="consts", bufs=1) as cpool,
            tc.tile_pool(name="inbf", bufs=6) as bfpool,
            tc.tile_pool(name="outp", bufs=6) as outpool,
            tc.tile_pool(name="mid", bufs=6) as midpool,
            tc.tile_pool(name="ps1", bufs=3, space="PSUM") as ps1pool,
            tc.tile_pool(name="ps2", bufs=3, space="PSUM") as ps2pool,
        ):
            bd_f = cpool.tile([128, 128], f32)
            m0_f = cpool.tile([128, 256], f32)
            m1_f = cpool.tile([128, 256], f32)
            nc.sync.dma_start(out=bd_f[:], in_=bd_d[:])
            nc.sync.dma_start(out=m0_f[:], in_=m0_d[:])
            nc.sync.dma_start(out=m1_f[:], in_=m1_d[:])
            bd_sb = cpool.tile([128, 128], bf16)
            nc.vector.tensor_copy(out=bd_sb[:], in_=bd_f[:])
            m0_sb = cpool.tile([128, 256], bf16)
            m1_sb = cpool.tile([128, 256], bf16)
            nc.vector.tensor_copy(out=m0_sb[:], in_=m0_f[:])
            nc.vector.tensor_copy(out=m1_sb[:], in_=m1_f[:])

            # Stores are issued 2 chunks late in program order so the Q7
            # SWDGE desc-gen never stalls waiting for compute (a stalled
            # store-gen would starve the load stream behind it).
            STORE_LAG = 2
            pending = []
            for cb in range(n_chunks):
                # SWDGE cast-during-DMA: reads f32 from HBM, lands bf16 in
                # SBUF (the cast runs inline in the SDMA engines).
                in_bf = bfpool.tile([128, G, 256], bf16)
                nc.gpsimd.dma_start(out=in_bf[:], in_=xv[cb])
                out_tile = outpool.tile([128, G, 256], bf16)
                for s in range(G // SB):
                    ps1 = ps1pool.tile([128, SB * 256], f32)
                    for g2 in range(SB):
                        g = s * SB + g2
                        for v in range(2):
                            nc.tensor.matmul(
                                ps1[:, g2 * 256 + v * 128 : g2 * 256 + (v + 1) * 128],
                                in_bf[:, g, v * 128 : (v + 1) * 128],
                                bd_sb[:],
                                start=True,
                                stop=True,
                            )
                    t1 = midpool.tile([128, SB * 256], bf16)
                    nc.scalar.copy(t1[:], ps1[:])
                    ps2 = ps2pool.tile([128, SB * 256], f32)
                    for g2 in range(SB):
                        nc.tensor.matmul(
                            ps2[:, g2 * 256 : (g2 + 1) * 256],
                            t1[:, g2 * 256 : g2 * 256 + 128],
                            m0_sb[:],
                            start=True,
                            stop=False,
                        )
                        nc.tensor.matmul(
                            ps2[:, g2 * 256 : (g2 + 1) * 256],
                            t1[:, g2 * 256 + 128 : g2 * 256 + 256],
                            m1_sb[:],
                            start=False,
                            stop=True,
                        )
                    nc.vector.tensor_copy(
                        out=out_tile[:, s * SB : (s + 1) * SB].rearrange(
                            "p g c -> p (g c)"
                        ),
                        in_=ps2[:],
                    )
                # SWDGE cast-during-DMA on the store too: bf16 in SBUF,
                # f32 in HBM (halves the SBUF-side S2M traffic).
                pending.append((cb, out_tile))
                if len(pending) > STORE_LAG:
                    scb, stile = pending.pop(0)
                    nc.gpsimd.dma_start(out=yv[scb], in_=stile[:])
            for scb, stile in pending:
                nc.gpsimd.dma_start(out=yv[scb], in_=stile[:])

    nc.compile()
    return nc


def _get_program():
    if "nc" not in _PROG_CACHE:
        _PROG_CACHE["nc"] = _build_program()
    return _PROG_CACHE["nc"]


def kernel(x, _trace=False, _trace_kwargs=None):
    from concourse.bass_utils import run_bass_kernel_spmd

    x = np.ascontiguousarray(np.asarray(x, dtype=np.float32))
    assert x.shape == (N_ROWS, DIM), x.shape

    nc = _get_program()
    core_ids = list(range(N_CORES))
    in_maps = [
        {"x": x[c * R_PER_CORE : (c + 1) * R_PER_CORE]} for c in core_ids
    ]
    res = run_bass_kernel_spmd(
        nc, in_maps, core_ids, trace=_trace, **(_trace_kwargs or {})
    )
    out = np.concatenate([r["y"] for r in res.results], axis=0)
    if _trace:
        return out, res
    return out
